# revision 10
# baseline (speedup 1.0000x reference)
"""Trainium2 Bass kernel for nn_MultiHeadHighLevelAllocator (v2: fused relu).

Math (reference):
    ue = MLP3(uav_feat)                            # (B,U,E)
    te = MLP3(task_feat)                           # (B,T,E)
    logits[b,h,u,t] = relu(ue[b,u]@Wq + head_q[h]@Wq + fb1
                           + te[b,t]@Wk) @ fw2 + fb2

Key decomposition (v2): fold the head bias into the task side once:
    khP4[d, (h,t)] = (te[b,t]@Wk)[d] + (head_q[h]@Wq + fb1)[d]
built ON THE PE as two accumulating matmuls (Wk-proj with the teT moving
operand broadcast over h, plus a rank-4 selector matmul adding hq rows).
Then the ENTIRE per-(u) elementwise work is ONE fused instruction:
    rt[d, (h,t)] = relu(khP4[d, (h,t)] + qP[d,u])       FD=512
with qP[d,u] = (ue[b,u]@Wq)[d] as the per-partition scalar operand.
No separate base materialization pass exists; DVE runs these at 4x mode
(~350ns) and ScalarE takes a ~1/3 share (~730ns), block-granular so each
consuming matmul needs a single sync wait.

Reduction: masked-stationary matmuls as v1: a (128x32) stationary holding
fw2-chunk c in column j writes dot products to PSUM partition 32g+j of
strip g's bank; moving is a 2D AP over the rt block (4 u's x 128 t at
head offset h*128). b=0/b=1 blocks are interleaved so the matmul stream
rotates over all four PE column groups.

Output: logits accumulate in 4 PSUM banks (32 partitions each) and are
DMAed straight from PSUM to HBM; fb2 is added host-side in the gather.
"""
import os
import sys

for _p in ("/opt/trn_rl_repo", "/root/.axon_site/_ro/trn_rl_repo"):
    if os.path.isdir(_p) and _p not in sys.path:
        sys.path.insert(0, _p)

import numpy as np
import concourse.bass as bass
import concourse.mybir as mybir
from concourse import tile

B, U, T = 16, 64, 128
UAV_DIM, TASK_DIM = 32, 32
E, H, HID = 128, 4, 256
ENC_H = 128
NCORES = 8
BL = B // NCORES          # batches per core
NBLK = U // 4             # u-blocks of 4
f32 = mybir.dt.float32
bf16 = mybir.dt.bfloat16
AF = mybir.ActivationFunctionType
ALU = mybir.AluOpType

# ---- packed fp32 tensor column layout ----
_C_UAVT = 0          # (32, 128)  uav features, transposed
_C_TASKT = 128       # (32, 256)  task features, transposed
_C_UW0 = 384         # (32, 128)
_C_TW0 = 512         # (32, 128)
_C_UW1 = 640         # (128, 128)
_C_UW2 = 768
_C_TW1 = 896
_C_TW2 = 1024
_C_ENCB = 1152       # (128, 6): ub0 ub1 ub2 tb0 tb1 tb2
_C32_TOTAL = 1158

# ---- packed bf16 tensor column layout ----
_B_WQK = 0           # (128, 512): Wq c0 | Wq c1 | Wk c0 | Wk c1
# wz: per c-chunk two 64-col segments (parity split so every masked
# stationary window starts 4B-aligned, keeping FWL on): segment A holds
# fw2-chunk c at col 31 (used for odd j, window [31-j, 63-j)); segment B
# holds it at col 32 (even j, window [32-j, 64-j)).
_B_WZ = 512          # (128, 256): c*128 + {A:0..63, B:64..127}
_B_HQ = 768          # (4, 256): hq[h, c*128+d] = (head_q@Wq + fb1)[h, c,d]
_B_SEL = 1024        # (4, 512): sel[h, h'*128+t] = (h == h')
_B_TOTAL = 1536

# relu engine split: ScalarE takes 19 of 64 blocks (Bresenham spread,
# phase-shifted so the first and last blocks run on the faster DVE).
ACT_SET = {i for i in range(64) if i % 10 in (2, 5, 8)}

_cache: dict = {}


def _split_multi_waits(nc):
    """Walrus rejects >1 sync wait per engine instruction. Hoist extra waits
    onto preceding same-engine NoOps - identical semantics on the in-order
    engine queues."""
    n_split = 0
    for func in nc.m.functions:
        for bb in func.blocks:
            new = []
            for ins in bb.instructions:
                si = ins.sync_info
                waits = list(si.on_wait) if (si and si.on_wait) else []
                if len(waits) > 1:
                    for k, w in enumerate(waits[:-1]):
                        nop = mybir.InstNoOp(name=f"{ins.name}_hw{k}", ins=[], outs=[])
                        nop.engine = ins.engine
                        nop.sync_info = mybir.SyncInfo(on_wait=[w], on_update=[])
                        new.append(nop)
                        n_split += 1
                    si.on_wait = [waits[-1]]
                new.append(ins)
            bb.instructions = new
    return n_split


def _strip_own_waits(nc):
    """Drop sem waits that only guard same-engine ordering on ScalarE/DVE.

    Those queues are strict-FIFO with a hardware pipe DRAIN between ops, so
    WAW/WAR hazards within one engine cannot occur; walrus still emits
    own-sem waits for tile-ring reuse, and each one costs a ~0.5us issue
    bubble (the queue must drain until its own completion count catches
    up). Safe only for engines that never reorder (NOT the PE, whose
    LDWEIGHTS can be pulled ahead) and only for monotonically-incremented
    sems owned entirely by that engine's synchronous instructions."""
    import collections
    eng_ok = set()
    for e in ("Activation", "DVE"):
        eng_ok.add(getattr(mybir.EngineType, e, None))
    sem_engines = collections.defaultdict(set)
    sem_pure = collections.defaultdict(lambda: True)
    prog = []
    for func in nc.m.functions:
        for bb in func.blocks:
            for ins in bb.instructions:
                prog.append(ins)
                si = ins.sync_info
                if si and si.on_update:
                    for upd in si.on_update:
                        if upd.sync_type != "semaphore":
                            continue
                        sem_engines[upd.id].add(ins.engine)
                        if upd.update_mode != "sem-inc":
                            sem_pure[upd.id] = False
                        if type(ins).__name__.startswith(("InstDMA",
                                                          "InstTensorLoad")):
                            sem_pure[upd.id] = False
    counts = collections.defaultdict(int)
    n_strip = 0
    for ins in prog:
        si = ins.sync_info
        if si and si.on_wait and ins.engine in eng_ok:
            kept = []
            for w in si.on_wait:
                if (w.sync_type == "semaphore"
                        and w.wait_mode == "sem-ge-imm"
                        and sem_pure[w.id]
                        and sem_engines[w.id] == {ins.engine}
                        and counts[w.id] >= w.wait_value):
                    n_strip += 1
                else:
                    kept.append(w)
            si.on_wait = kept
        if si and si.on_update:
            for upd in si.on_update:
                if upd.sync_type == "semaphore" \
                        and upd.update_mode == "sem-inc":
                    counts[upd.id] += upd.update_value
    return n_strip


def _build_nc():
    nc = bass.Bass()
    p32 = nc.dram_tensor("p32", [128, _C32_TOTAL], f32, kind="ExternalInput")
    p16 = nc.dram_tensor("p16", [128, _B_TOTAL], bf16, kind="ExternalInput")
    out = nc.dram_tensor("out", [128, 512], f32, kind="ExternalOutput")

    with tile.TileContext(nc) as tc:
        with (
            tc.tile_pool(name="const", bufs=1) as constp,
            tc.tile_pool(name="persist", bufs=1) as persistp,
            tc.tile_pool(name="encw", bufs=2) as encwp,
        ):
            A = constp.tile([128, _C32_TOTAL], f32, tag="a32")
            Bt = constp.tile([128, _B_TOTAL], bf16, tag="a16")
            # phase-ordered loads so the encoders start as early as possible
            nc.sync.dma_start(A[:, :640], p32[:, :640])              # in+l1 w
            nc.sync.dma_start(A[:, 640:], p32[:, 640:])              # enc w+b
            nc.sync.dma_start(Bt[:, :], p16[:, :])                   # proj w
            # first-touch of DMA'd tiles per engine so later instructions
            # never pair a DMA-sem wait with an engine-sem wait.
            act_touch = constp.tile([128, 2], f32, tag="acttouch")
            nc.scalar.copy(act_touch[:, 0:1], A[:, 0:1])
            nc.scalar.copy(act_touch[:, 1:2], Bt[:, 0:1])
            dve_touch = constp.tile([128, 2], f32, tag="dvetouch")
            nc.vector.tensor_copy(dve_touch[:, 0:1], A[:, 640:641])
            nc.vector.tensor_copy(dve_touch[:, 1:2], Bt[:, 0:1])

            enc_w = {
                "uw0": A[0:32, _C_UW0:_C_UW0 + 128],
                "tw0": A[0:32, _C_TW0:_C_TW0 + 128],
                "uw1": A[:, _C_UW1:_C_UW1 + 128],
                "uw2": A[:, _C_UW2:_C_UW2 + 128],
                "tw1": A[:, _C_TW1:_C_TW1 + 128],
                "tw2": A[:, _C_TW2:_C_TW2 + 128],
            }

            def encb_col(i):
                return A[:, _C_ENCB + i:_C_ENCB + i + 1]

            with (
                tc.tile_pool(name="relupv", bufs=6) as relupv,
                tc.tile_pool(name="relupa", bufs=4) as relupa,
                tc.tile_pool(name="workp", bufs=2, space="PSUM") as psW,
                tc.tile_pool(name="qpp", bufs=2, space="PSUM") as psQ,
                tc.tile_pool(name="lpp", bufs=1, space="PSUM") as psL,
            ):
                # ---- PE warm-up: ~3.5us of dependency-free matmuls during
                # the DMA window so HAM un-throttles (4/8 -> 8/8) before the
                # encoder chain needs the PE. ----
                warm_w = constp.tile([128, 32], bf16, tag="warmw")
                warm_m = constp.tile([128, 512], bf16, tag="warmm")
                nc.gpsimd.memset(warm_w[:], 0.0)
                nc.gpsimd.memset(warm_m[:], 0.0)
                warm_ps = psW.tile([128, 512], f32, tag="wk", name="warm")
                for _ in range(9):
                    nc.tensor.matmul(warm_ps[0:32, :], warm_w[:], warm_m[:],
                                     start=True, stop=True)
                # ---- encoders: ue/te chains interleaved; bf16 throughout ----
                chains = {
                    "ue": [A[0:32, _C_UAVT:_C_UAVT + BL * U], BL * U,
                           ("uw0", "uw1", "uw2"), (0, 1, 2)],
                    "te": [A[0:32, _C_TASKT:_C_TASKT + BL * T], BL * T,
                           ("tw0", "tw1", "tw2"), (3, 4, 5)],
                }
                cur = {k: v[0] for k, v in chains.items()}
                for li in range(3):
                    pss = {}
                    for k, (x0, rows, wn, bc) in chains.items():
                        ps = psW.tile([128, 512], f32, tag="wk",
                                      name=f"ps{k}{li}")
                        nc.tensor.matmul(ps[:, :rows], enc_w[wn[li]], cur[k],
                                         start=True, stop=True)
                        pss[k] = ps
                    for k, (x0, rows, wn, bc) in chains.items():
                        if li < 2:
                            nxt = encwp.tile([128, rows], f32, tag=f"{k}h",
                                             name=f"{k}h{li}")
                            if k == "ue":
                                nc.vector.tensor_scalar(
                                    nxt[:], pss[k][:, :rows],
                                    encb_col(bc[li]), 0.0, ALU.add, ALU.max)
                            else:
                                nc.scalar.activation(nxt[:], pss[k][:, :rows],
                                                     AF.Relu,
                                                     bias=encb_col(bc[li]),
                                                     scale=1.0)
                        else:
                            nxt = persistp.tile([128, rows], bf16, tag=f"{k}T",
                                                name=f"{k}T")
                            if k == "ue":
                                nc.vector.tensor_scalar(
                                    nxt[:], pss[k][:, :rows],
                                    encb_col(bc[li]), None, ALU.add)
                            else:
                                nc.scalar.activation(nxt[:], pss[k][:, :rows],
                                                     AF.Identity,
                                                     bias=encb_col(bc[li]),
                                                     scale=1.0)
                        cur[k] = nxt[:]
                ueT, teT = cur["ue"], cur["te"]

                # ---- khP4 + qP preludes ----
                # khP4[d,(h,t)] = Wk_c-proj(te_b) bcast over h + hq rows
                khP4s, qPs = {}, {}
                sel = Bt[0:4, _B_SEL:_B_SEL + 512]

                def prelude(b, c):
                    pk = psW.tile([128, 512], f32, tag="wk",
                                  name=f"pk{b}{c}")
                    mov = teT[:, b * T:(b + 1) * T].unsqueeze(1) \
                        .broadcast_to([128, H, T])
                    nc.tensor.matmul(pk[:, :512],
                                     Bt[:, _B_WQK + 256 + c * 128:
                                        _B_WQK + 256 + (c + 1) * 128],
                                     mov, start=True, stop=False)
                    nc.tensor.matmul(pk[:, :512],
                                     Bt[0:4, _B_HQ + c * 128:
                                        _B_HQ + (c + 1) * 128],
                                     sel, start=False, stop=True)
                    khP4 = persistp.tile([128, 512], bf16,
                                         tag=f"khP4{b}{c}",
                                         name=f"khP4{b}{c}")
                    nc.vector.tensor_copy(khP4[:], pk[:, :512])
                    khP4s[(b, c)] = khP4

                    pq = psQ.tile([128, 64], f32, tag="qp", name=f"pq{b}{c}")
                    nc.tensor.matmul(pq[:, :U],
                                     Bt[:, _B_WQK + c * 128:
                                        _B_WQK + (c + 1) * 128],
                                     ueT[:, b * U:(b + 1) * U],
                                     start=True, stop=True)
                    qP = persistp.tile([128, U], f32, tag=f"qP{b}{c}",
                                       name=f"qP{b}{c}")
                    nc.scalar.activation(qP[:], pq[:, :U], AF.Identity,
                                         bias=0.0, scale=1.0)
                    qPs[(b, c)] = qP

                # ---- main loop: fused relu blocks + reduction matmuls ----
                # rtblk [128, 2048] per (b,c,n): col = du*512 + h*128 + t.
                # The relu for u=4n+du writes contiguous 512 cols; the
                # reduction moving for h is a 2D AP (du: step 512 x4,
                # t: step 1 x128) at base h*128. Blocks alternate engines
                # (separate pools keep ring WAW deps within one engine).
                # b-major: strips g0/g1 (b=0) finish halfway through, so
                # their output half is copied + DMAed while b=1 streams.
                lp = [psL.tile([128, 512], f32, tag=f"lp{g}", name=f"lp{g}")
                      for g in range(4)]
                sb_out = persistp.tile([128, 512], f32, tag="sbout",
                                       name="sbout")

                def do_block(b, c, n, bi):
                    use_act = bi in ACT_SET
                    pool = relupa if use_act else relupv
                    rtblk = pool.tile([128, 2048], bf16, tag="rt", name="rt")
                    khP4, qP = khP4s[(b, c)], qPs[(b, c)]
                    for du in range(4):
                        u = 4 * n + du
                        dst = rtblk[:, du * 512:(du + 1) * 512]
                        if use_act:
                            nc.scalar.activation(
                                dst, khP4[:], AF.Relu,
                                bias=qP[:, u:u + 1], scale=1.0)
                        else:
                            nc.vector.tensor_scalar(
                                dst, khP4[:], qP[:, u:u + 1], 0.0,
                                ALU.add, ALU.max)
                    rt3 = rtblk[:].rearrange("p (du x) -> p du x", du=4)
                    for h in (0, 2, 1, 3):
                        p_ = (b * H + h) * NBLK + n
                        g, j = p_ // 32, p_ % 32
                        first = (c == 0 and n == 0 and h == 2 * (g % 2))
                        last = (c == 1 and n == NBLK - 1
                                and h == 2 * (g % 2) + 1)
                        if j % 2 == 1:
                            wst = Bt[:, _B_WZ + c * 128 + 31 - j:
                                     _B_WZ + c * 128 + 63 - j]
                        else:
                            wst = Bt[:, _B_WZ + c * 128 + 96 - j:
                                     _B_WZ + c * 128 + 128 - j]
                        nc.tensor.matmul(
                            lp[g][32 * g:32 * g + 32, :],
                            wst, rt3[:, :, h * 128:(h + 1) * 128],
                            start=first, stop=last,
                            tile_position=(0, 32 * g))

                def flush_half(b):
                    for g in (2 * b, 2 * b + 1):
                        dst = sb_out[32 * g:32 * g + 32, :]
                        src = lp[g][32 * g:32 * g + 32, :]
                        if g % 2 == 0:
                            nc.vector.tensor_copy(dst, src)
                        else:
                            nc.scalar.copy(dst, src)
                    nc.sync.dma_start(out[64 * b:64 * b + 64, :],
                                      sb_out[64 * b:64 * b + 64, :])

                # b=0 preludes, two blocks to get the stream going, then the
                # remaining preludes slot in behind them.
                prelude(0, 0)
                bi = 0
                for n in range(2):
                    do_block(0, 0, n, bi)
                    bi += 1
                prelude(0, 1)
                prelude(1, 0)
                prelude(1, 1)
                for n in range(2, NBLK):
                    do_block(0, 0, n, bi)
                    bi += 1
                for n in range(NBLK):
                    do_block(0, 1, n, bi)
                    bi += 1
                flush_half(0)
                for c in range(2):
                    for n in range(NBLK):
                        do_block(1, c, n, bi)
                        bi += 1
                flush_half(1)
    return nc


def _prep_inputs(uav_feat, task_feat, uw0, ub0, uw1, ub1, uw2, ub2,
                 tw0, tb0, tw1, tb1, tw2, tb2, head_q, fw1, fb1, fw2, fb2):
    import ml_dtypes
    f = np.float32
    uav = np.asarray(uav_feat, f)
    task = np.asarray(task_feat, f)
    fw1 = np.asarray(fw1, f)
    fw2 = np.asarray(fw2, f)
    Wq, Wk = fw1[:E], fw1[E:]

    b32 = np.zeros((128, _C32_TOTAL), f)
    b32[0:32, _C_UW0:_C_UW0 + 128] = np.asarray(uw0, f)
    b32[0:32, _C_TW0:_C_TW0 + 128] = np.asarray(tw0, f)
    b32[:, _C_UW1:_C_UW1 + 128] = np.asarray(uw1, f)
    b32[:, _C_UW2:_C_UW2 + 128] = np.asarray(uw2, f)
    b32[:, _C_TW1:_C_TW1 + 128] = np.asarray(tw1, f)
    b32[:, _C_TW2:_C_TW2 + 128] = np.asarray(tw2, f)
    for i, v in enumerate((ub0, ub1, ub2, tb0, tb1, tb2)):
        b32[:, _C_ENCB + i] = np.asarray(v, f)

    b16 = np.zeros((128, _B_TOTAL), f)
    b16[:, _B_WQK:_B_WQK + 256] = Wq
    b16[:, _B_WQK + 256:_B_WQK + 512] = Wk
    for c in range(2):
        b16[:, _B_WZ + c * 128 + 31] = fw2[c * 128:(c + 1) * 128, 0]
        b16[:, _B_WZ + c * 128 + 64 + 32] = fw2[c * 128:(c + 1) * 128, 0]
    hq = np.asarray(head_q, f) @ Wq + np.asarray(fb1, f)  # (H, HID)
    b16[0:4, _B_HQ:_B_HQ + 256] = hq  # col c*128+d == contiguous HID
    for h in range(H):
        b16[h, _B_SEL + h * 128:_B_SEL + (h + 1) * 128] = 1.0

    b16c = b16.astype(ml_dtypes.bfloat16)
    in_maps = []
    for k in range(NCORES):
        b0 = k * BL
        pk32 = b32.copy()
        pk32[0:32, _C_UAVT:_C_UAVT + BL * U] = \
            uav[b0:b0 + BL].reshape(BL * U, UAV_DIM).T
        pk32[0:32, _C_TASKT:_C_TASKT + BL * T] = \
            task[b0:b0 + BL].reshape(BL * T, TASK_DIM).T
        in_maps.append({"p32": pk32, "p16": b16c})
    return in_maps


def _gather(results, fb2):
    fb2v = float(np.asarray(fb2, np.float32)[0])
    outs = []
    for k in range(NCORES):
        r = np.asarray(results[k]["out"], np.float32)  # (128, 512)
        outs.append(r.reshape(BL, H, NBLK, 4, T).reshape(BL, H, U, T))
    return np.concatenate(outs, axis=0) + fb2v


def kernel(**inputs) -> np.ndarray:
    if "nc" not in _cache:
        _cache["nc"] = _build_nc()
    nc = _cache["nc"]
    in_maps = _prep_inputs(**inputs)
    if os.environ.get("BASS_KERNEL_SIM"):
        from concourse.bass_interp import CoreSim
        results = []
        for k in range(NCORES):
            sim = CoreSim(nc)
            for name, arr in in_maps[k].items():
                sim.tensor(name)[:] = arr
            sim.simulate()
            results.append({"out": np.array(sim.tensor("out"))})
    else:
        from concourse.bass_utils import run_bass_kernel_spmd
        if not _cache.get("split"):
            _strip_own_waits(nc)
            _split_multi_waits(nc)
            _cache["split"] = True
        results = run_bass_kernel_spmd(nc, in_maps, list(range(NCORES))).results
    return _gather(results, inputs["fb2"])


# revision 11
# speedup vs baseline: 1.0007x; 1.0007x over previous
"""Trainium2 Bass kernel for nn_MultiHeadHighLevelAllocator (v2: fused relu).

Math (reference):
    ue = MLP3(uav_feat)                            # (B,U,E)
    te = MLP3(task_feat)                           # (B,T,E)
    logits[b,h,u,t] = relu(ue[b,u]@Wq + head_q[h]@Wq + fb1
                           + te[b,t]@Wk) @ fw2 + fb2

Key decomposition (v2): fold the head bias into the task side once:
    khP4[d, (h,t)] = (te[b,t]@Wk)[d] + (head_q[h]@Wq + fb1)[d]
built ON THE PE as two accumulating matmuls (Wk-proj with the teT moving
operand broadcast over h, plus a rank-4 selector matmul adding hq rows).
Then the ENTIRE per-(u) elementwise work is ONE fused instruction:
    rt[d, (h,t)] = relu(khP4[d, (h,t)] + qP[d,u])       FD=512
with qP[d,u] = (ue[b,u]@Wq)[d] as the per-partition scalar operand.
No separate base materialization pass exists; DVE runs these at 4x mode
(~350ns) and ScalarE takes a ~1/3 share (~730ns), block-granular so each
consuming matmul needs a single sync wait.

Reduction: masked-stationary matmuls as v1: a (128x32) stationary holding
fw2-chunk c in column j writes dot products to PSUM partition 32g+j of
strip g's bank; moving is a 2D AP over the rt block (4 u's x 128 t at
head offset h*128). b=0/b=1 blocks are interleaved so the matmul stream
rotates over all four PE column groups.

Output: logits accumulate in 4 PSUM banks (32 partitions each) and are
DMAed straight from PSUM to HBM; fb2 is added host-side in the gather.
"""
import os
import sys

for _p in ("/opt/trn_rl_repo", "/root/.axon_site/_ro/trn_rl_repo"):
    if os.path.isdir(_p) and _p not in sys.path:
        sys.path.insert(0, _p)

import numpy as np
import concourse.bass as bass
import concourse.mybir as mybir
from concourse import tile

B, U, T = 16, 64, 128
UAV_DIM, TASK_DIM = 32, 32
E, H, HID = 128, 4, 256
ENC_H = 128
NCORES = 8
BL = B // NCORES          # batches per core
NBLK = U // 4             # u-blocks of 4
f32 = mybir.dt.float32
bf16 = mybir.dt.bfloat16
AF = mybir.ActivationFunctionType
ALU = mybir.AluOpType

# ---- packed fp32 tensor column layout ----
_C_UAVT = 0          # (32, 128)  uav features, transposed
_C_TASKT = 128       # (32, 256)  task features, transposed
_C_UW0 = 384         # (32, 128)
_C_TW0 = 512         # (32, 128)
_C_UW1 = 640         # (128, 128)
_C_UW2 = 768
_C_TW1 = 896
_C_TW2 = 1024
_C_ENCB = 1152       # (128, 6): ub0 ub1 ub2 tb0 tb1 tb2
_C32_TOTAL = 1158

# ---- packed bf16 tensor column layout ----
_B_WQK = 0           # (128, 512): Wq c0 | Wq c1 | Wk c0 | Wk c1
# wz: per c-chunk two 64-col segments (parity split so every masked
# stationary window starts 4B-aligned, keeping FWL on): segment A holds
# fw2-chunk c at col 31 (used for odd j, window [31-j, 63-j)); segment B
# holds it at col 32 (even j, window [32-j, 64-j)).
_B_WZ = 512          # (128, 256): c*128 + {A:0..63, B:64..127}
_B_HQ = 768          # (4, 256): hq[h, c*128+d] = (head_q@Wq + fb1)[h, c,d]
_B_SEL = 1024        # (4, 512): sel[h, h'*128+t] = (h == h')
_B_TOTAL = 1536

# relu engine split: ScalarE takes 19 of 64 blocks (Bresenham spread,
# phase-shifted so the first and last blocks run on the faster DVE).
ACT_SET = {i for i in range(64) if i % 10 in (2, 5, 8)}

_cache: dict = {}


def _split_multi_waits(nc):
    """Walrus rejects >1 sync wait per engine instruction. Hoist extra waits
    onto preceding same-engine NoOps - identical semantics on the in-order
    engine queues."""
    n_split = 0
    for func in nc.m.functions:
        for bb in func.blocks:
            new = []
            for ins in bb.instructions:
                si = ins.sync_info
                waits = list(si.on_wait) if (si and si.on_wait) else []
                if len(waits) > 1:
                    for k, w in enumerate(waits[:-1]):
                        nop = mybir.InstNoOp(name=f"{ins.name}_hw{k}", ins=[], outs=[])
                        nop.engine = ins.engine
                        nop.sync_info = mybir.SyncInfo(on_wait=[w], on_update=[])
                        new.append(nop)
                        n_split += 1
                    si.on_wait = [waits[-1]]
                new.append(ins)
            bb.instructions = new
    return n_split


def _strip_own_waits(nc):
    """Drop sem waits that only guard same-engine ordering on ScalarE/DVE.

    Those queues are strict-FIFO with a hardware pipe DRAIN between ops, so
    WAW/WAR hazards within one engine cannot occur; walrus still emits
    own-sem waits for tile-ring reuse, and each one costs a ~0.5us issue
    bubble (the queue must drain until its own completion count catches
    up). Safe only for engines that never reorder (NOT the PE, whose
    LDWEIGHTS can be pulled ahead) and only for monotonically-incremented
    sems owned entirely by that engine's synchronous instructions."""
    import collections
    eng_ok = set()
    for e in ("Activation", "DVE"):
        eng_ok.add(getattr(mybir.EngineType, e, None))
    sem_engines = collections.defaultdict(set)
    sem_pure = collections.defaultdict(lambda: True)
    prog = []
    for func in nc.m.functions:
        for bb in func.blocks:
            for ins in bb.instructions:
                prog.append(ins)
                si = ins.sync_info
                if si and si.on_update:
                    for upd in si.on_update:
                        if upd.sync_type != "semaphore":
                            continue
                        sem_engines[upd.id].add(ins.engine)
                        if upd.update_mode != "sem-inc":
                            sem_pure[upd.id] = False
                        if type(ins).__name__.startswith(("InstDMA",
                                                          "InstTensorLoad")):
                            sem_pure[upd.id] = False
    counts = collections.defaultdict(int)
    n_strip = 0
    for ins in prog:
        si = ins.sync_info
        if si and si.on_wait and ins.engine in eng_ok:
            kept = []
            for w in si.on_wait:
                if (w.sync_type == "semaphore"
                        and w.wait_mode == "sem-ge-imm"
                        and sem_pure[w.id]
                        and sem_engines[w.id] == {ins.engine}
                        and counts[w.id] >= w.wait_value):
                    n_strip += 1
                else:
                    kept.append(w)
            si.on_wait = kept
        if si and si.on_update:
            for upd in si.on_update:
                if upd.sync_type == "semaphore" \
                        and upd.update_mode == "sem-inc":
                    counts[upd.id] += upd.update_value
    return n_strip


def _build_nc():
    nc = bass.Bass()
    p32 = nc.dram_tensor("p32", [128, _C32_TOTAL], f32, kind="ExternalInput")
    p16 = nc.dram_tensor("p16", [128, _B_TOTAL], bf16, kind="ExternalInput")
    out = nc.dram_tensor("out", [128, 512], f32, kind="ExternalOutput")

    with tile.TileContext(nc) as tc:
        with (
            tc.tile_pool(name="const", bufs=1) as constp,
            tc.tile_pool(name="persist", bufs=1) as persistp,
            tc.tile_pool(name="encw", bufs=2) as encwp,
        ):
            A = constp.tile([128, _C32_TOTAL], f32, tag="a32")
            Bt = constp.tile([128, _B_TOTAL], bf16, tag="a16")
            # phase-ordered loads so the encoders start as early as possible
            nc.sync.dma_start(A[:, :640], p32[:, :640])              # in+l1 w
            nc.sync.dma_start(A[:, 640:], p32[:, 640:])              # enc w+b
            nc.sync.dma_start(Bt[:, :], p16[:, :])                   # proj w
            # first-touch of DMA'd tiles per engine so later instructions
            # never pair a DMA-sem wait with an engine-sem wait.
            act_touch = constp.tile([128, 2], f32, tag="acttouch")
            nc.scalar.copy(act_touch[:, 0:1], A[:, 0:1])
            nc.scalar.copy(act_touch[:, 1:2], Bt[:, 0:1])
            dve_touch = constp.tile([128, 2], f32, tag="dvetouch")
            nc.vector.tensor_copy(dve_touch[:, 0:1], A[:, 640:641])
            nc.vector.tensor_copy(dve_touch[:, 1:2], Bt[:, 0:1])

            enc_w = {
                "uw0": A[0:32, _C_UW0:_C_UW0 + 128],
                "tw0": A[0:32, _C_TW0:_C_TW0 + 128],
                "uw1": A[:, _C_UW1:_C_UW1 + 128],
                "uw2": A[:, _C_UW2:_C_UW2 + 128],
                "tw1": A[:, _C_TW1:_C_TW1 + 128],
                "tw2": A[:, _C_TW2:_C_TW2 + 128],
            }

            def encb_col(i):
                return A[:, _C_ENCB + i:_C_ENCB + i + 1]

            with (
                tc.tile_pool(name="relupv", bufs=6) as relupv,
                tc.tile_pool(name="relupa", bufs=4) as relupa,
                tc.tile_pool(name="workp", bufs=2, space="PSUM") as psW,
                tc.tile_pool(name="qpp", bufs=2, space="PSUM") as psQ,
                tc.tile_pool(name="lpp", bufs=1, space="PSUM") as psL,
            ):
                # ---- PE warm-up: ~3.5us of dependency-free matmuls during
                # the DMA window so HAM un-throttles (4/8 -> 8/8) before the
                # encoder chain needs the PE. ----
                warm_w = constp.tile([128, 32], bf16, tag="warmw")
                warm_m = constp.tile([128, 512], bf16, tag="warmm")
                nc.gpsimd.memset(warm_w[:], 0.0)
                nc.gpsimd.memset(warm_m[:], 0.0)
                warm_ps = psW.tile([128, 512], f32, tag="wk", name="warm")
                for _ in range(9):
                    nc.tensor.matmul(warm_ps[0:32, :], warm_w[:], warm_m[:],
                                     start=True, stop=True)
                # ---- encoders: ue/te chains interleaved; bf16 throughout ----
                chains = {
                    "ue": [A[0:32, _C_UAVT:_C_UAVT + BL * U], BL * U,
                           ("uw0", "uw1", "uw2"), (0, 1, 2)],
                    "te": [A[0:32, _C_TASKT:_C_TASKT + BL * T], BL * T,
                           ("tw0", "tw1", "tw2"), (3, 4, 5)],
                }
                cur = {k: v[0] for k, v in chains.items()}
                for li in range(3):
                    pss = {}
                    for k, (x0, rows, wn, bc) in chains.items():
                        ps = psW.tile([128, 512], f32, tag="wk",
                                      name=f"ps{k}{li}")
                        nc.tensor.matmul(ps[:, :rows], enc_w[wn[li]], cur[k],
                                         start=True, stop=True)
                        pss[k] = ps
                    for k, (x0, rows, wn, bc) in chains.items():
                        if li < 2:
                            nxt = encwp.tile([128, rows], f32, tag=f"{k}h",
                                             name=f"{k}h{li}")
                            if k == "ue":
                                nc.vector.tensor_scalar(
                                    nxt[:], pss[k][:, :rows],
                                    encb_col(bc[li]), 0.0, ALU.add, ALU.max)
                            else:
                                nc.scalar.activation(nxt[:], pss[k][:, :rows],
                                                     AF.Relu,
                                                     bias=encb_col(bc[li]),
                                                     scale=1.0)
                        else:
                            nxt = persistp.tile([128, rows], bf16, tag=f"{k}T",
                                                name=f"{k}T")
                            if k == "ue":
                                nc.vector.tensor_scalar(
                                    nxt[:], pss[k][:, :rows],
                                    encb_col(bc[li]), None, ALU.add)
                            else:
                                nc.scalar.activation(nxt[:], pss[k][:, :rows],
                                                     AF.Identity,
                                                     bias=encb_col(bc[li]),
                                                     scale=1.0)
                        cur[k] = nxt[:]
                ueT, teT = cur["ue"], cur["te"]

                # ---- khP4 + qP preludes ----
                # khP4[d,(h,t)] = Wk_c-proj(te_b) bcast over h + hq rows
                khP4s, qPs = {}, {}
                sel = Bt[0:4, _B_SEL:_B_SEL + 512]

                def prelude(b, c):
                    pk = psW.tile([128, 512], f32, tag="wk",
                                  name=f"pk{b}{c}")
                    mov = teT[:, b * T:(b + 1) * T].unsqueeze(1) \
                        .broadcast_to([128, H, T])
                    nc.tensor.matmul(pk[:, :512],
                                     Bt[:, _B_WQK + 256 + c * 128:
                                        _B_WQK + 256 + (c + 1) * 128],
                                     mov, start=True, stop=False)
                    nc.tensor.matmul(pk[:, :512],
                                     Bt[0:4, _B_HQ + c * 128:
                                        _B_HQ + (c + 1) * 128],
                                     sel, start=False, stop=True)
                    khP4 = persistp.tile([128, 512], bf16,
                                         tag=f"khP4{b}{c}",
                                         name=f"khP4{b}{c}")
                    nc.vector.tensor_copy(khP4[:], pk[:, :512])
                    khP4s[(b, c)] = khP4

                    pq = psQ.tile([128, 64], f32, tag="qp", name=f"pq{b}{c}")
                    nc.tensor.matmul(pq[:, :U],
                                     Bt[:, _B_WQK + c * 128:
                                        _B_WQK + (c + 1) * 128],
                                     ueT[:, b * U:(b + 1) * U],
                                     start=True, stop=True)
                    qP = persistp.tile([128, U], f32, tag=f"qP{b}{c}",
                                       name=f"qP{b}{c}")
                    nc.scalar.activation(qP[:], pq[:, :U], AF.Identity,
                                         bias=0.0, scale=1.0)
                    qPs[(b, c)] = qP

                # ---- main loop: fused relu blocks + reduction matmuls ----
                # rtblk [128, 2048] per (b,c,n): col = du*512 + h*128 + t.
                # The relu for u=4n+du writes contiguous 512 cols; the
                # reduction moving for h is a 2D AP (du: step 512 x4,
                # t: step 1 x128) at base h*128. Blocks alternate engines
                # (separate pools keep ring WAW deps within one engine).
                # b-major: strips g0/g1 (b=0) finish halfway through, so
                # their output half is copied + DMAed while b=1 streams.
                lp = [psL.tile([128, 512], f32, tag=f"lp{g}", name=f"lp{g}")
                      for g in range(4)]
                sb_out = persistp.tile([128, 512], f32, tag="sbout",
                                       name="sbout")

                def do_block(b, c, n, bi):
                    use_act = bi in ACT_SET
                    pool = relupa if use_act else relupv
                    rtblk = pool.tile([128, 2048], bf16, tag="rt", name="rt")
                    khP4, qP = khP4s[(b, c)], qPs[(b, c)]
                    for du in range(4):
                        u = 4 * n + du
                        dst = rtblk[:, du * 512:(du + 1) * 512]
                        if use_act:
                            nc.scalar.activation(
                                dst, khP4[:], AF.Relu,
                                bias=qP[:, u:u + 1], scale=1.0)
                        else:
                            nc.vector.tensor_scalar(
                                dst, khP4[:], qP[:, u:u + 1], 0.0,
                                ALU.add, ALU.max)
                    rt3 = rtblk[:].rearrange("p (du x) -> p du x", du=4)
                    for h in (0, 2, 1, 3):
                        p_ = (b * H + h) * NBLK + n
                        g, j = p_ // 32, p_ % 32
                        first = (c == 0 and n == 0 and h == 2 * (g % 2))
                        last = (c == 1 and n == NBLK - 1
                                and h == 2 * (g % 2) + 1)
                        if j % 2 == 1:
                            wst = Bt[:, _B_WZ + c * 128 + 31 - j:
                                     _B_WZ + c * 128 + 63 - j]
                        else:
                            wst = Bt[:, _B_WZ + c * 128 + 96 - j:
                                     _B_WZ + c * 128 + 128 - j]
                        nc.tensor.matmul(
                            lp[g][32 * g:32 * g + 32, :],
                            wst, rt3[:, :, h * 128:(h + 1) * 128],
                            start=first, stop=last,
                            tile_position=(0, 32 * g))

                def flush_half(b):
                    for g in (2 * b, 2 * b + 1):
                        dst = sb_out[32 * g:32 * g + 32, :]
                        src = lp[g][32 * g:32 * g + 32, :]
                        if g % 2 == 0:
                            nc.vector.tensor_copy(dst, src)
                        else:
                            nc.scalar.copy(dst, src)
                    nc.sync.dma_start(out[64 * b:64 * b + 64, :],
                                      sb_out[64 * b:64 * b + 64, :])

                # b=0 preludes, two blocks to get the stream going, then the
                # remaining preludes slot in behind them.
                prelude(0, 0)
                bi = 0
                for n in range(2):
                    do_block(0, 0, n, bi)
                    bi += 1
                prelude(0, 1)
                prelude(1, 0)
                prelude(1, 1)
                for n in range(2, NBLK):
                    do_block(0, 0, n, bi)
                    bi += 1
                for n in range(NBLK):
                    do_block(0, 1, n, bi)
                    bi += 1
                flush_half(0)
                for c in range(2):
                    for n in range(NBLK):
                        do_block(1, c, n, bi)
                        bi += 1
                flush_half(1)
    return nc


def _prep_inputs(uav_feat, task_feat, uw0, ub0, uw1, ub1, uw2, ub2,
                 tw0, tb0, tw1, tb1, tw2, tb2, head_q, fw1, fb1, fw2, fb2):
    import ml_dtypes
    f = np.float32
    uav = np.asarray(uav_feat, f)
    task = np.asarray(task_feat, f)
    fw1 = np.asarray(fw1, f)
    fw2 = np.asarray(fw2, f)
    Wq, Wk = fw1[:E], fw1[E:]

    b32 = np.zeros((128, _C32_TOTAL), f)
    b32[0:32, _C_UW0:_C_UW0 + 128] = np.asarray(uw0, f)
    b32[0:32, _C_TW0:_C_TW0 + 128] = np.asarray(tw0, f)
    b32[:, _C_UW1:_C_UW1 + 128] = np.asarray(uw1, f)
    b32[:, _C_UW2:_C_UW2 + 128] = np.asarray(uw2, f)
    b32[:, _C_TW1:_C_TW1 + 128] = np.asarray(tw1, f)
    b32[:, _C_TW2:_C_TW2 + 128] = np.asarray(tw2, f)
    for i, v in enumerate((ub0, ub1, ub2, tb0, tb1, tb2)):
        b32[:, _C_ENCB + i] = np.asarray(v, f)

    b16 = np.zeros((128, _B_TOTAL), f)
    b16[:, _B_WQK:_B_WQK + 256] = Wq
    b16[:, _B_WQK + 256:_B_WQK + 512] = Wk
    for c in range(2):
        b16[:, _B_WZ + c * 128 + 31] = fw2[c * 128:(c + 1) * 128, 0]
        b16[:, _B_WZ + c * 128 + 64 + 32] = fw2[c * 128:(c + 1) * 128, 0]
    hq = np.asarray(head_q, f) @ Wq + np.asarray(fb1, f)  # (H, HID)
    b16[0:4, _B_HQ:_B_HQ + 256] = hq  # col c*128+d == contiguous HID
    for h in range(H):
        b16[h, _B_SEL + h * 128:_B_SEL + (h + 1) * 128] = 1.0

    b16c = b16.astype(ml_dtypes.bfloat16)
    in_maps = []
    for k in range(NCORES):
        b0 = k * BL
        pk32 = b32.copy()
        pk32[0:32, _C_UAVT:_C_UAVT + BL * U] = \
            uav[b0:b0 + BL].reshape(BL * U, UAV_DIM).T
        pk32[0:32, _C_TASKT:_C_TASKT + BL * T] = \
            task[b0:b0 + BL].reshape(BL * T, TASK_DIM).T
        in_maps.append({"p32": pk32, "p16": b16c})
    return in_maps


def _gather(results, fb2):
    fb2v = float(np.asarray(fb2, np.float32)[0])
    outs = []
    for k in range(NCORES):
        r = np.asarray(results[k]["out"], np.float32)  # (128, 512)
        outs.append(r.reshape(BL, H, NBLK, 4, T).reshape(BL, H, U, T))
    return np.concatenate(outs, axis=0) + fb2v


def kernel(**inputs) -> np.ndarray:
    if "nc" not in _cache:
        _cache["nc"] = _build_nc()
    nc = _cache["nc"]
    in_maps = _prep_inputs(**inputs)
    if os.environ.get("BASS_KERNEL_SIM"):
        from concourse.bass_interp import CoreSim
        results = []
        for k in range(NCORES):
            sim = CoreSim(nc)
            for name, arr in in_maps[k].items():
                sim.tensor(name)[:] = arr
            sim.simulate()
            results.append({"out": np.array(sim.tensor("out"))})
    else:
        from concourse.bass_utils import run_bass_kernel_spmd
        if not _cache.get("split"):
            # NOTE: _strip_own_waits measured 2.8us SLOWER on HW despite
            # removing 56 redundant-looking waits - the producer engines
            # run ahead and hit cross-engine waits in a worse pattern.
            # Left available but intentionally not applied.
            _split_multi_waits(nc)
            _cache["split"] = True
        results = run_bass_kernel_spmd(nc, in_maps, list(range(NCORES))).results
    return _gather(results, inputs["fb2"])


# revision 12
# speedup vs baseline: 1.0459x; 1.0452x over previous
"""Trainium2 Bass kernel for nn_MultiHeadHighLevelAllocator (v2: fused relu).

Math (reference):
    ue = MLP3(uav_feat)                            # (B,U,E)
    te = MLP3(task_feat)                           # (B,T,E)
    logits[b,h,u,t] = relu(ue[b,u]@Wq + head_q[h]@Wq + fb1
                           + te[b,t]@Wk) @ fw2 + fb2

Key decomposition (v2): fold the head bias into the task side once:
    khP4[d, (h,t)] = (te[b,t]@Wk)[d] + (head_q[h]@Wq + fb1)[d]
built ON THE PE as two accumulating matmuls (Wk-proj with the teT moving
operand broadcast over h, plus a rank-4 selector matmul adding hq rows).
Then the ENTIRE per-(u) elementwise work is ONE fused instruction:
    rt[d, (h,t)] = relu(khP4[d, (h,t)] + qP[d,u])       FD=512
with qP[d,u] = (ue[b,u]@Wq)[d] as the per-partition scalar operand.
No separate base materialization pass exists; DVE runs these at 4x mode
(~346ns) and ScalarE takes 19/64 blocks (~700ns each), block-granular so
each consuming matmul needs a single sync wait.

Reduction: masked-stationary matmuls as v1: a (128x32) stationary holding
fw2-chunk c in column j writes dot products to PSUM partition 32g+j of
strip g's bank; moving is a 2D AP over the rt block (4 u's x 128 t at
head offset h*128). b=0/b=1 blocks are interleaved so the matmul stream
rotates over all four PE column groups.

Output: logits accumulate in 4 PSUM banks (32 partitions each); each
64-partition half is copied to SBUF and DMAed as soon as its b-section
finishes (b-major loop order). fb2 is added host-side in the gather.
A ~3.5us burst of dependency-free warm-up matmuls during the DMA window
un-throttles the PE clock (HAM 4/8 -> 8/8) before the encoders run.
"""
import os
import sys

for _p in ("/opt/trn_rl_repo", "/root/.axon_site/_ro/trn_rl_repo"):
    if os.path.isdir(_p) and _p not in sys.path:
        sys.path.insert(0, _p)

import numpy as np
import concourse.bass as bass
import concourse.mybir as mybir
from concourse import tile

B, U, T = 16, 64, 128
UAV_DIM, TASK_DIM = 32, 32
E, H, HID = 128, 4, 256
ENC_H = 128
NCORES = 8
BL = B // NCORES          # batches per core
NBLK = U // 4             # u-blocks of 4
f32 = mybir.dt.float32
bf16 = mybir.dt.bfloat16
AF = mybir.ActivationFunctionType
ALU = mybir.AluOpType

# ---- packed fp32 tensor column layout ----
_C_UAVT = 0          # (32, 128)  uav features, transposed
_C_TASKT = 128       # (32, 256)  task features, transposed
_C_UW0 = 384         # (32, 128)
_C_TW0 = 512         # (32, 128)
_C_UW1 = 640         # (128, 128)
_C_UW2 = 768
_C_TW1 = 896
_C_TW2 = 1024
_C_ENCB = 1152       # (128, 6): ub0 ub1 ub2 tb0 tb1 tb2
_C32_TOTAL = 1158

# ---- packed bf16 tensor column layout ----
_B_WQK = 0           # (128, 512): Wq c0 | Wq c1 | Wk c0 | Wk c1
# wz: per c-chunk two 64-col segments (parity split so every masked
# stationary window starts 4B-aligned, keeping FWL on): segment A holds
# fw2-chunk c at col 31 (used for odd j, window [31-j, 63-j)); segment B
# holds it at col 32 (even j, window [32-j, 64-j)).
_B_WZ = 512          # (128, 256): c*128 + {A:0..63, B:64..127}
_B_HQ = 768          # (4, 256): hq[h, c*128+d] = (head_q@Wq + fb1)[h, c,d]
_B_SEL = 1024        # (4, 512): sel[h, h'*128+t] = (h == h')
_B_TOTAL = 1536

# relu engine split: ScalarE takes 19 of 64 blocks (Bresenham spread,
# phase-shifted so the first and last blocks run on the faster DVE).
ACT_SET = {i for i in range(64) if i % 10 in (2, 5, 8)}

_cache: dict = {}


def _split_multi_waits(nc):
    """Walrus rejects >1 sync wait per engine instruction. Hoist extra waits
    onto preceding same-engine NoOps - identical semantics on the in-order
    engine queues."""
    n_split = 0
    for func in nc.m.functions:
        for bb in func.blocks:
            new = []
            for ins in bb.instructions:
                si = ins.sync_info
                waits = list(si.on_wait) if (si and si.on_wait) else []
                if len(waits) > 1:
                    for k, w in enumerate(waits[:-1]):
                        nop = mybir.InstNoOp(name=f"{ins.name}_hw{k}", ins=[], outs=[])
                        nop.engine = ins.engine
                        nop.sync_info = mybir.SyncInfo(on_wait=[w], on_update=[])
                        new.append(nop)
                        n_split += 1
                    si.on_wait = [waits[-1]]
                new.append(ins)
            bb.instructions = new
    return n_split


def _strip_own_waits(nc):
    """Drop sem waits that only guard same-engine ordering on ScalarE/DVE.

    Those queues are strict-FIFO with a hardware pipe DRAIN between ops, so
    WAW/WAR hazards within one engine cannot occur; walrus still emits
    own-sem waits for tile-ring reuse, and each one costs a ~0.5us issue
    bubble (the queue must drain until its own completion count catches
    up). Safe only for engines that never reorder (NOT the PE, whose
    LDWEIGHTS can be pulled ahead) and only for monotonically-incremented
    sems owned entirely by that engine's synchronous instructions."""
    import collections
    eng_ok = set()
    for e in ("Activation", "DVE"):
        eng_ok.add(getattr(mybir.EngineType, e, None))
    sem_engines = collections.defaultdict(set)
    sem_pure = collections.defaultdict(lambda: True)
    prog = []
    for func in nc.m.functions:
        for bb in func.blocks:
            for ins in bb.instructions:
                prog.append(ins)
                si = ins.sync_info
                if si and si.on_update:
                    for upd in si.on_update:
                        if upd.sync_type != "semaphore":
                            continue
                        sem_engines[upd.id].add(ins.engine)
                        if upd.update_mode != "sem-inc":
                            sem_pure[upd.id] = False
                        if type(ins).__name__.startswith(("InstDMA",
                                                          "InstTensorLoad")):
                            sem_pure[upd.id] = False
    counts = collections.defaultdict(int)
    n_strip = 0
    for ins in prog:
        si = ins.sync_info
        if si and si.on_wait and ins.engine in eng_ok:
            kept = []
            for w in si.on_wait:
                if (w.sync_type == "semaphore"
                        and w.wait_mode == "sem-ge-imm"
                        and sem_pure[w.id]
                        and sem_engines[w.id] == {ins.engine}
                        and counts[w.id] >= w.wait_value):
                    n_strip += 1
                else:
                    kept.append(w)
            si.on_wait = kept
        if si and si.on_update:
            for upd in si.on_update:
                if upd.sync_type == "semaphore" \
                        and upd.update_mode == "sem-inc":
                    counts[upd.id] += upd.update_value
    return n_strip


def _build_nc():
    nc = bass.Bass()
    p32 = nc.dram_tensor("p32", [128, _C32_TOTAL], f32, kind="ExternalInput")
    p16 = nc.dram_tensor("p16", [128, _B_TOTAL], bf16, kind="ExternalInput")
    out = nc.dram_tensor("out", [128, 512], f32, kind="ExternalOutput")

    with tile.TileContext(nc) as tc:
        with (
            tc.tile_pool(name="const", bufs=1) as constp,
            tc.tile_pool(name="persist", bufs=1) as persistp,
            tc.tile_pool(name="encw", bufs=2) as encwp,
        ):
            A = constp.tile([128, _C32_TOTAL], f32, tag="a32")
            Bt = constp.tile([128, _B_TOTAL], bf16, tag="a16")
            # phase-ordered loads so the encoders start as early as possible
            nc.sync.dma_start(A[:, :640], p32[:, :640])              # in+l1 w
            nc.sync.dma_start(A[:, 640:], p32[:, 640:])              # enc w+b
            nc.sync.dma_start(Bt[:, :], p16[:, :])                   # proj w
            # first-touch of DMA'd tiles per engine so later instructions
            # never pair a DMA-sem wait with an engine-sem wait.
            act_touch = constp.tile([128, 2], f32, tag="acttouch")
            nc.scalar.copy(act_touch[:, 0:1], A[:, 0:1])
            nc.scalar.copy(act_touch[:, 1:2], Bt[:, 0:1])
            dve_touch = constp.tile([128, 2], f32, tag="dvetouch")
            nc.vector.tensor_copy(dve_touch[:, 0:1], A[:, 640:641])
            nc.vector.tensor_copy(dve_touch[:, 1:2], Bt[:, 0:1])

            enc_w = {
                "uw0": A[0:32, _C_UW0:_C_UW0 + 128],
                "tw0": A[0:32, _C_TW0:_C_TW0 + 128],
                "uw1": A[:, _C_UW1:_C_UW1 + 128],
                "uw2": A[:, _C_UW2:_C_UW2 + 128],
                "tw1": A[:, _C_TW1:_C_TW1 + 128],
                "tw2": A[:, _C_TW2:_C_TW2 + 128],
            }

            def encb_col(i):
                return A[:, _C_ENCB + i:_C_ENCB + i + 1]

            with (
                tc.tile_pool(name="relupv", bufs=6) as relupv,
                tc.tile_pool(name="relupa", bufs=4) as relupa,
                tc.tile_pool(name="workp", bufs=2, space="PSUM") as psW,
                tc.tile_pool(name="qpp", bufs=2, space="PSUM") as psQ,
                tc.tile_pool(name="lpp", bufs=1, space="PSUM") as psL,
            ):
                # ---- PE warm-up: ~3.5us of dependency-free matmuls during
                # the DMA window so HAM un-throttles (4/8 -> 8/8) before the
                # encoder chain needs the PE. ----
                warm_w = constp.tile([128, 32], bf16, tag="warmw")
                warm_m = constp.tile([128, 512], bf16, tag="warmm")
                nc.gpsimd.memset(warm_w[:], 0.0)
                nc.gpsimd.memset(warm_m[:], 0.0)
                warm_ps = psW.tile([128, 512], f32, tag="wk", name="warm")
                for _ in range(9):
                    nc.tensor.matmul(warm_ps[0:32, :], warm_w[:], warm_m[:],
                                     start=True, stop=True)
                # ---- encoders: ue/te chains interleaved; bf16 throughout ----
                chains = {
                    "ue": [A[0:32, _C_UAVT:_C_UAVT + BL * U], BL * U,
                           ("uw0", "uw1", "uw2"), (0, 1, 2)],
                    "te": [A[0:32, _C_TASKT:_C_TASKT + BL * T], BL * T,
                           ("tw0", "tw1", "tw2"), (3, 4, 5)],
                }
                cur = {k: v[0] for k, v in chains.items()}
                for li in range(3):
                    pss = {}
                    for k, (x0, rows, wn, bc) in chains.items():
                        ps = psW.tile([128, 512], f32, tag="wk",
                                      name=f"ps{k}{li}")
                        nc.tensor.matmul(ps[:, :rows], enc_w[wn[li]], cur[k],
                                         start=True, stop=True)
                        pss[k] = ps
                    for k, (x0, rows, wn, bc) in chains.items():
                        if li < 2:
                            nxt = encwp.tile([128, rows], f32, tag=f"{k}h",
                                             name=f"{k}h{li}")
                            if k == "ue":
                                nc.vector.tensor_scalar(
                                    nxt[:], pss[k][:, :rows],
                                    encb_col(bc[li]), 0.0, ALU.add, ALU.max)
                            else:
                                nc.scalar.activation(nxt[:], pss[k][:, :rows],
                                                     AF.Relu,
                                                     bias=encb_col(bc[li]),
                                                     scale=1.0)
                        else:
                            nxt = persistp.tile([128, rows], bf16, tag=f"{k}T",
                                                name=f"{k}T")
                            if k == "ue":
                                nc.vector.tensor_scalar(
                                    nxt[:], pss[k][:, :rows],
                                    encb_col(bc[li]), None, ALU.add)
                            else:
                                nc.scalar.activation(nxt[:], pss[k][:, :rows],
                                                     AF.Identity,
                                                     bias=encb_col(bc[li]),
                                                     scale=1.0)
                        cur[k] = nxt[:]
                ueT, teT = cur["ue"], cur["te"]

                # ---- khP4 + qP preludes ----
                # khP4[d,(h,t)] = Wk_c-proj(te_b) bcast over h + hq rows
                khP4s, qPs = {}, {}
                sel = Bt[0:4, _B_SEL:_B_SEL + 512]

                def prelude(b, c):
                    pk = psW.tile([128, 512], f32, tag="wk",
                                  name=f"pk{b}{c}")
                    mov = teT[:, b * T:(b + 1) * T].unsqueeze(1) \
                        .broadcast_to([128, H, T])
                    nc.tensor.matmul(pk[:, :512],
                                     Bt[:, _B_WQK + 256 + c * 128:
                                        _B_WQK + 256 + (c + 1) * 128],
                                     mov, start=True, stop=False)
                    nc.tensor.matmul(pk[:, :512],
                                     Bt[0:4, _B_HQ + c * 128:
                                        _B_HQ + (c + 1) * 128],
                                     sel, start=False, stop=True)
                    khP4 = persistp.tile([128, 512], bf16,
                                         tag=f"khP4{b}{c}",
                                         name=f"khP4{b}{c}")
                    nc.vector.tensor_copy(khP4[:], pk[:, :512])
                    khP4s[(b, c)] = khP4

                    pq = psQ.tile([128, 64], f32, tag="qp", name=f"pq{b}{c}")
                    nc.tensor.matmul(pq[:, :U],
                                     Bt[:, _B_WQK + c * 128:
                                        _B_WQK + (c + 1) * 128],
                                     ueT[:, b * U:(b + 1) * U],
                                     start=True, stop=True)
                    qP = persistp.tile([128, U], f32, tag=f"qP{b}{c}",
                                       name=f"qP{b}{c}")
                    nc.scalar.activation(qP[:], pq[:, :U], AF.Identity,
                                         bias=0.0, scale=1.0)
                    qPs[(b, c)] = qP

                # ---- main loop: fused relu blocks + reduction matmuls ----
                # rtblk [128, 2048] per (b,c,n): col = du*512 + h*128 + t.
                # The relu for u=4n+du writes contiguous 512 cols; the
                # reduction moving for h is a 2D AP (du: step 512 x4,
                # t: step 1 x128) at base h*128. Blocks alternate engines
                # (separate pools keep ring WAW deps within one engine).
                # b-major: strips g0/g1 (b=0) finish halfway through, so
                # their output half is copied + DMAed while b=1 streams.
                lp = [psL.tile([128, 512], f32, tag=f"lp{g}", name=f"lp{g}")
                      for g in range(4)]
                sb_out = persistp.tile([128, 512], f32, tag="sbout",
                                       name="sbout")

                def do_block(b, c, n, bi):
                    use_act = bi in ACT_SET
                    pool = relupa if use_act else relupv
                    rtblk = pool.tile([128, 2048], bf16, tag="rt", name="rt")
                    khP4, qP = khP4s[(b, c)], qPs[(b, c)]
                    for du in range(4):
                        u = 4 * n + du
                        dst = rtblk[:, du * 512:(du + 1) * 512]
                        if use_act:
                            nc.scalar.activation(
                                dst, khP4[:], AF.Relu,
                                bias=qP[:, u:u + 1], scale=1.0)
                        else:
                            nc.vector.tensor_scalar(
                                dst, khP4[:], qP[:, u:u + 1], 0.0,
                                ALU.add, ALU.max)
                    rt3 = rtblk[:].rearrange("p (du x) -> p du x", du=4)
                    for h in (0, 2, 1, 3):
                        p_ = (b * H + h) * NBLK + n
                        g, j = p_ // 32, p_ % 32
                        first = (c == 0 and n == 0 and h == 2 * (g % 2))
                        last = (c == 1 and n == NBLK - 1
                                and h == 2 * (g % 2) + 1)
                        if j % 2 == 1:
                            wst = Bt[:, _B_WZ + c * 128 + 31 - j:
                                     _B_WZ + c * 128 + 63 - j]
                        else:
                            wst = Bt[:, _B_WZ + c * 128 + 96 - j:
                                     _B_WZ + c * 128 + 128 - j]
                        nc.tensor.matmul(
                            lp[g][32 * g:32 * g + 32, :],
                            wst, rt3[:, :, h * 128:(h + 1) * 128],
                            start=first, stop=last,
                            tile_position=(0, 32 * g))

                def flush_half(b):
                    for g in (2 * b, 2 * b + 1):
                        dst = sb_out[32 * g:32 * g + 32, :]
                        src = lp[g][32 * g:32 * g + 32, :]
                        if g % 2 == 0:
                            nc.vector.tensor_copy(dst, src)
                        else:
                            nc.scalar.copy(dst, src)
                    nc.sync.dma_start(out[64 * b:64 * b + 64, :],
                                      sb_out[64 * b:64 * b + 64, :])

                # b=0 preludes, two blocks to get the stream going, then the
                # remaining preludes slot in behind them.
                prelude(0, 0)
                bi = 0
                for n in range(2):
                    do_block(0, 0, n, bi)
                    bi += 1
                prelude(0, 1)
                prelude(1, 0)
                prelude(1, 1)
                for n in range(2, NBLK):
                    do_block(0, 0, n, bi)
                    bi += 1
                for n in range(NBLK):
                    do_block(0, 1, n, bi)
                    bi += 1
                flush_half(0)
                for c in range(2):
                    for n in range(NBLK):
                        do_block(1, c, n, bi)
                        bi += 1
                flush_half(1)
    return nc


def _prep_inputs(uav_feat, task_feat, uw0, ub0, uw1, ub1, uw2, ub2,
                 tw0, tb0, tw1, tb1, tw2, tb2, head_q, fw1, fb1, fw2, fb2):
    import ml_dtypes
    f = np.float32
    uav = np.asarray(uav_feat, f)
    task = np.asarray(task_feat, f)
    fw1 = np.asarray(fw1, f)
    fw2 = np.asarray(fw2, f)
    Wq, Wk = fw1[:E], fw1[E:]

    b32 = np.zeros((128, _C32_TOTAL), f)
    b32[0:32, _C_UW0:_C_UW0 + 128] = np.asarray(uw0, f)
    b32[0:32, _C_TW0:_C_TW0 + 128] = np.asarray(tw0, f)
    b32[:, _C_UW1:_C_UW1 + 128] = np.asarray(uw1, f)
    b32[:, _C_UW2:_C_UW2 + 128] = np.asarray(uw2, f)
    b32[:, _C_TW1:_C_TW1 + 128] = np.asarray(tw1, f)
    b32[:, _C_TW2:_C_TW2 + 128] = np.asarray(tw2, f)
    for i, v in enumerate((ub0, ub1, ub2, tb0, tb1, tb2)):
        b32[:, _C_ENCB + i] = np.asarray(v, f)

    b16 = np.zeros((128, _B_TOTAL), f)
    b16[:, _B_WQK:_B_WQK + 256] = Wq
    b16[:, _B_WQK + 256:_B_WQK + 512] = Wk
    for c in range(2):
        b16[:, _B_WZ + c * 128 + 31] = fw2[c * 128:(c + 1) * 128, 0]
        b16[:, _B_WZ + c * 128 + 64 + 32] = fw2[c * 128:(c + 1) * 128, 0]
    hq = np.asarray(head_q, f) @ Wq + np.asarray(fb1, f)  # (H, HID)
    b16[0:4, _B_HQ:_B_HQ + 256] = hq  # col c*128+d == contiguous HID
    for h in range(H):
        b16[h, _B_SEL + h * 128:_B_SEL + (h + 1) * 128] = 1.0

    b16c = b16.astype(ml_dtypes.bfloat16)
    in_maps = []
    for k in range(NCORES):
        b0 = k * BL
        pk32 = b32.copy()
        pk32[0:32, _C_UAVT:_C_UAVT + BL * U] = \
            uav[b0:b0 + BL].reshape(BL * U, UAV_DIM).T
        pk32[0:32, _C_TASKT:_C_TASKT + BL * T] = \
            task[b0:b0 + BL].reshape(BL * T, TASK_DIM).T
        in_maps.append({"p32": pk32, "p16": b16c})
    return in_maps


def _gather(results, fb2):
    fb2v = float(np.asarray(fb2, np.float32)[0])
    outs = []
    for k in range(NCORES):
        r = np.asarray(results[k]["out"], np.float32)  # (128, 512)
        outs.append(r.reshape(BL, H, NBLK, 4, T).reshape(BL, H, U, T))
    return np.concatenate(outs, axis=0) + fb2v


def kernel(**inputs) -> np.ndarray:
    if "nc" not in _cache:
        _cache["nc"] = _build_nc()
    nc = _cache["nc"]
    in_maps = _prep_inputs(**inputs)
    if os.environ.get("BASS_KERNEL_SIM"):
        from concourse.bass_interp import CoreSim
        results = []
        for k in range(NCORES):
            sim = CoreSim(nc)
            for name, arr in in_maps[k].items():
                sim.tensor(name)[:] = arr
            sim.simulate()
            results.append({"out": np.array(sim.tensor("out"))})
    else:
        from concourse.bass_utils import run_bass_kernel_spmd
        if not _cache.get("split"):
            # NOTE: _strip_own_waits measured 2.8us SLOWER on HW despite
            # removing 56 redundant-looking waits - the producer engines
            # run ahead and hit cross-engine waits in a worse pattern.
            # Left available but intentionally not applied.
            _split_multi_waits(nc)
            _cache["split"] = True
        results = run_bass_kernel_spmd(nc, in_maps, list(range(NCORES))).results
    return _gather(results, inputs["fb2"])
